# revision 1
# baseline (speedup 1.0000x reference)
"""GammaScorer edge-scoring kernel for 8 Trainium2 NeuronCores.

score[e] = sigmoid((x[src[e]] * x[dst[e]]) @ W.T + b)

Strategy: shard the 640K edges across 8 cores (80K each); replicate the
node table x and the tiny linear weights on every core. Per core, gather
src/dst rows from HBM with indirect DMAs (one row index per SBUF
partition -> 128 rows of 512B per instruction), then do the elementwise
product + W-weighted reduction on the vector engine, sigmoid on the
scalar engine.

Host-side index layout: a core's 80000 edges are reshaped to [128, 625]
C-order, so partition p holds edges p*625..p*625+624 and the final
[128, 625] score tile stores to DRAM contiguously (no transposes).
"""

import sys

import numpy as np

sys.path.insert(0, "/opt/trn_rl_repo")

N_NODES = 100000
D = 128
E = 640000
N_CORES = 8
P = 128
EPC = E // N_CORES          # 80000 edges per core
COLS = EPC // P             # 625 edge-columns per partition
K = 5                       # edge-columns per macro-tile -> 125 iterations

_NC_CACHE = {}


def _build_nc():
    if "nc" in _NC_CACHE:
        return _NC_CACHE["nc"]

    from contextlib import ExitStack

    import concourse.bacc as bacc
    import concourse.bass as bass
    import concourse.tile as tile
    from concourse import mybir

    f32 = mybir.dt.float32
    i32 = mybir.dt.int32

    nc = bacc.Bacc(
        "TRN2",
        target_bir_lowering=False,
        debug=False,
        num_devices=N_CORES,
    )
    x = nc.dram_tensor("x", [N_NODES, D], f32, kind="ExternalInput")
    src = nc.dram_tensor("src", [P, COLS], i32, kind="ExternalInput")
    dst = nc.dram_tensor("dst", [P, COLS], i32, kind="ExternalInput")
    wrep = nc.dram_tensor("wrep", [P, K * D], f32, kind="ExternalInput")
    brep = nc.dram_tensor("brep", [P, 1], f32, kind="ExternalInput")
    out = nc.dram_tensor("out", [P, COLS], f32, kind="ExternalOutput")

    with tile.TileContext(nc) as tc, ExitStack() as ctx:
        const = ctx.enter_context(tc.tile_pool(name="const", bufs=1))
        work = ctx.enter_context(tc.tile_pool(name="work", bufs=3))
        res = ctx.enter_context(tc.tile_pool(name="res", bufs=1))

        w_sb = const.tile([P, K * D], f32)
        nc.sync.dma_start(w_sb[:], wrep[:])
        b_sb = const.tile([P, 1], f32)
        nc.sync.dma_start(b_sb[:], brep[:])
        src_sb = const.tile([P, COLS], i32)
        nc.sync.dma_start(src_sb[:], src[:])
        dst_sb = const.tile([P, COLS], i32)
        nc.sync.dma_start(dst_sb[:], dst[:])

        scores = res.tile([P, COLS], f32)

        for i in range(COLS // K):
            s_tile = work.tile([P, K * D], f32, tag="S")
            t_tile = work.tile([P, K * D], f32, tag="T")
            for k in range(K):
                c = i * K + k
                nc.gpsimd.indirect_dma_start(
                    out=s_tile[:, k * D : (k + 1) * D],
                    out_offset=None,
                    in_=x[:],
                    in_offset=bass.IndirectOffsetOnAxis(
                        ap=src_sb[:, c : c + 1], axis=0
                    ),
                )
                nc.gpsimd.indirect_dma_start(
                    out=t_tile[:, k * D : (k + 1) * D],
                    out_offset=None,
                    in_=x[:],
                    in_offset=bass.IndirectOffsetOnAxis(
                        ap=dst_sb[:, c : c + 1], axis=0
                    ),
                )
            u_tile = work.tile([P, K * D], f32, tag="U")
            nc.vector.tensor_mul(u_tile[:], s_tile[:], t_tile[:])
            v_tile = work.tile([P, K * D], f32, tag="V")
            nc.vector.tensor_mul(v_tile[:], u_tile[:], w_sb[:])
            dots = work.tile([P, K], f32, tag="dots")
            nc.vector.reduce_sum(
                dots[:],
                v_tile[:].rearrange("p (k d) -> p k d", d=D),
                axis=mybir.AxisListType.X,
            )
            nc.scalar.activation(
                scores[:, i * K : (i + 1) * K],
                dots[:],
                mybir.ActivationFunctionType.Sigmoid,
                bias=b_sb[:],
            )

        nc.sync.dma_start(out[:], scores[:])

    nc.compile()
    _NC_CACHE["nc"] = nc
    return nc


def _prep_in_maps(x, src_idx, dst_idx, W, b):
    x = np.ascontiguousarray(np.asarray(x), dtype=np.float32)
    src_idx = np.asarray(src_idx)
    dst_idx = np.asarray(dst_idx)
    W = np.asarray(W, dtype=np.float32)
    b = np.asarray(b, dtype=np.float32)

    wrep = np.ascontiguousarray(np.tile(W.reshape(1, D), (P, K)))
    brep = np.full((P, 1), b.reshape(-1)[0], dtype=np.float32)

    in_maps = []
    for c in range(N_CORES):
        sl = slice(c * EPC, (c + 1) * EPC)
        in_maps.append(
            {
                "x": x,
                "src": np.ascontiguousarray(
                    src_idx[sl].astype(np.int32).reshape(P, COLS)
                ),
                "dst": np.ascontiguousarray(
                    dst_idx[sl].astype(np.int32).reshape(P, COLS)
                ),
                "wrep": wrep,
                "brep": brep,
            }
        )
    return in_maps


_last_in_maps = None


def kernel(x, src_idx, dst_idx, W, b):
    from concourse.bass_utils import run_bass_kernel_spmd

    nc = _build_nc()
    in_maps = _prep_in_maps(x, src_idx, dst_idx, W, b)

    global _last_in_maps
    _last_in_maps = in_maps

    results = run_bass_kernel_spmd(nc, in_maps, list(range(N_CORES))).results
    out = np.concatenate([r["out"].reshape(-1) for r in results])
    return out.reshape(E, 1).astype(np.float32)

